# revision 41
# baseline (speedup 1.0000x reference)
"""Paged-attention prefill kernel for Trainium2, tensor-parallel over heads on 8 cores.

Reference semantics (see problem): full prefill GQA attention with RoPE and a
paged KV cache. B=4 seqs x 1024 tokens, D=4096, 32 q heads / 8 kv heads, H=128.

Sharding: core c owns q heads 4c..4c+3 and kv head c. x is replicated (passed
pre-transposed as xT), w_q/w_o sharded on N, w_k/w_v on K. Each core computes
its 4 heads' attention and a partial O projection [T, D]; the host sums the 8
partials. k/v (post-RoPE) are returned per-core in token order; the host
scatters them into the paged caches via slot_mapping.

Device layout choices:
  - projections produce q/k/v in [t, h] layout (moving operand = weights,
    N=512/256), RoPE applied with free-dim strided views (no partition moves),
    then PE-transposes give qT/kT in [h, t] for attention.
  - scores are computed transposed: per 128-token q-tile, ONE matmul produces
    sT [s-tile, 4 heads x 128 q] with the 4 heads side by side in the moving
    dim (strided view of qT), so each LDWEIGHTS serves 4 heads' work. exp
    needs no max subtraction (scores ~ N(0,1), no overflow); causality: s-tiles
    above the diagonal are skipped, the diagonal tile is masked by a 0/1
    multiply on DVE after exp.
  - softmax denominator Z = ones[128,128]^T @ p accumulated in PSUM (M=128 so
    it pipelines like a normal matmul; rows are replicated so no partition
    broadcast is needed), then outT *= reciprocal_approx_fast(Z).
  - O projection keeps w_o stationary (reused across both 512-token moving
    chunks) and emits oT [D, T] partials in bf16; the host sums 8 partials in
    fp32 and transposes once.
  - all inputs are host-swizzled into the exact SBUF layouts so every DMA is
    partition-contiguous; leading x/weight DMAs are split fine-grained and
    spread across HWDGE queues so the first matmul starts at ~10us.
Compute dtype bf16 (fp32 PSUM accumulation).
"""
import numpy as np
import ml_dtypes

import concourse.bass as bass
import concourse.bacc as bacc
import concourse.mybir as mybir
import concourse.tile as tile
from concourse import bass_utils

F32 = mybir.dt.float32
BF16 = mybir.dt.bfloat16
AF = mybir.ActivationFunctionType
OP = mybir.AluOpType

NCORES = 8


class Cfg:
    def __init__(self, D=4096, T=4096, SQ=1024, NH=4, CHUNK=512):
        self.D, self.T, self.SQ, self.NH, self.CHUNK = D, T, SQ, NH, CHUNK
        self.H = 128
        self.n_dtile = D // 128
        self.n_ttile = T // 128
        self.n_seq = T // SQ
        self.tt_per_seq = SQ // 128
        self.st_per_chunk = CHUNK // 128
        self.chunks_per_seq = SQ // CHUNK
        self.DC = D // 512  # d-chunks for O projection
        assert SQ % CHUNK == 0 and CHUNK % 128 == 0 and D % 512 == 0


def build_kernel(cfg: Cfg):
    D, T, SQ, NH, CHUNK = cfg.D, cfg.T, cfg.SQ, cfg.NH, cfg.CHUNK
    H = cfg.H
    QN = NH * H  # q projection width per core (4 heads -> 512)

    nc = bacc.Bacc("TRN2", target_bir_lowering=False, debug=False)

    # ---- DRAM I/O (per-core), all pre-swizzled on host for contiguous DMA ----
    x_d = nc.dram_tensor("xt", [cfg.n_ttile, 128, cfg.n_dtile * 128], BF16,
                         kind="ExternalInput")
    wq_d = nc.dram_tensor("wq", [128, cfg.n_dtile * QN], BF16,
                          kind="ExternalInput")
    wkv_d = nc.dram_tensor("wkv", [128, cfg.n_dtile * 2 * H], BF16,
                           kind="ExternalInput")
    wo_d = nc.dram_tensor("wo", [128, NH * cfg.DC * 512], BF16,
                          kind="ExternalInput")
    cos_d = nc.dram_tensor("cosm", [128, cfg.n_ttile * 64], F32,
                           kind="ExternalInput")
    sin_d = nc.dram_tensor("sinm", [128, cfg.n_ttile * 64], F32,
                           kind="ExternalInput")
    mask_d = nc.dram_tensor("maskS", [128, NH * 128], BF16,
                            kind="ExternalInput")

    k_out = nc.dram_tensor("k_out", [T, H], F32, kind="ExternalOutput")
    v_out = nc.dram_tensor("v_out", [T, H], F32, kind="ExternalOutput")
    o_out = nc.dram_tensor("ot_part", [D, T], BF16, kind="ExternalOutput")

    with tile.TileContext(nc) as tc:
        # persistent constants in raw SBUF (concrete addresses -> manual
        # broadcast APs are safe)
        cos_sb = nc.alloc_sbuf_tensor("cos_sb", [128, cfg.n_ttile * 64], F32)
        sin_sb = nc.alloc_sbuf_tensor("sin_sb", [128, cfg.n_ttile * 64], F32)

        def rope_tab(sb, tt, nbc):
            """[128, nbc(bcast), 64] view of table column block tt."""
            ap = sb.ap()
            if nbc == 1:
                return ap[:, tt * 64:(tt + 1) * 64]
            return bass.AP(tensor=ap.tensor, offset=ap.offset + tt * 64,
                           ap=[list(ap.ap[0]), [0, nbc], [1, 64]])

        mask_sb = nc.alloc_sbuf_tensor("mask_sb", [128, NH * 128], BF16)
        ones_sb = nc.alloc_sbuf_tensor("ones_sb", [128, 128], BF16)

        with tc.tile_pool(name="persist", bufs=1) as persist, \
             tc.tile_pool(name="weights", bufs=1) as wpool:
            # resident weights
            wq_sb = wpool.tile([128, cfg.n_dtile, QN], BF16, tag="wq")
            wkv_sb = wpool.tile([128, cfg.n_dtile, 2 * H], BF16, tag="wkv")

            # attention operand stores (full T)
            qT_sb = persist.tile([128, NH, T], BF16, tag="qT")
            kT_sb = persist.tile([128, T], BF16, tag="kT")
            vb_sb = persist.tile([128, cfg.n_ttile, H], BF16, tag="vb")

            # ---------------- projection + RoPE + transposes ----------------
            with tc.tile_pool(name="proj_sb", bufs=4) as psb, \
                 tc.tile_pool(name="rope_sb", bufs=3) as rsb, \
                 tc.tile_pool(name="stage_sb", bufs=3) as ssb, \
                 tc.tile_pool(name="proj_ps", bufs=2, space="PSUM") as pps:
                def load_x(tt, steps=None, eng=None):
                    x_sb = psb.tile([128, cfg.n_dtile, 128], BF16, tag="x")
                    base = max(cfg.n_dtile // 4, 1)
                    sched = list(steps or [])
                    q4 = 0
                    while q4 < cfg.n_dtile:
                        step = min(sched.pop(0) if sched else base,
                                   cfg.n_dtile - q4)
                        (eng or nc.sync).dma_start(
                            out=x_sb[:, q4:q4 + step, :],
                            in_=x_d.ap()[tt, :, q4 * 128:(q4 + step) * 128])
                        q4 += step
                    return x_sb

                # startup order: x0, then weights chunk-interleaved, then consts
                x_pre = load_x(0, steps=[2, 2, 4], eng=nc.scalar)
                wsched = [1, 1, 2] if cfg.n_dtile >= 8 else []
                kq = 0
                while kq < cfg.n_dtile:
                    step = min(wsched.pop(0) if wsched else 4,
                               cfg.n_dtile - kq)
                    nc.sync.dma_start(
                        out=wq_sb[:, kq:kq + step, :],
                        in_=wq_d.ap()[:, kq * QN:(kq + step) * QN])
                    nc.sync.dma_start(
                        out=wkv_sb[:, kq:kq + step, :],
                        in_=wkv_d.ap()[:, kq * 2 * H:(kq + step) * 2 * H])
                    kq += step
                nc.scalar.dma_start(out=cos_sb.ap(), in_=cos_d.ap())
                nc.scalar.dma_start(out=sin_sb.ap(), in_=sin_d.ap())
                nc.scalar.dma_start(out=mask_sb.ap(), in_=mask_d.ap())
                nc.vector.memset(ones_sb.ap(), 1.0)

                def proj_epilogue(tt, ps_q, ps_kv):
                    # --- RoPE on q (all NH heads at once via strided views) ---
                    q3 = ps_q.rearrange("p (g x) -> p g x", g=NH)
                    cosb = rope_tab(cos_sb, tt, NH)
                    sinb = rope_tab(sin_sb, tt, NH)
                    t1 = rsb.tile([128, NH, H], F32, tag="t1")
                    t2 = rsb.tile([128, NH, H], F32, tag="t2")
                    nc.vector.tensor_mul(t1[:, :, 0:64], q3[:, :, 0:64], cosb)
                    nc.vector.tensor_mul(t1[:, :, 64:128], q3[:, :, 64:128], cosb)
                    nc.vector.scalar_tensor_tensor(
                        out=t2[:, :, 0:64], in0=q3[:, :, 64:128], scalar=-1.0,
                        in1=sinb, op0=OP.mult, op1=OP.mult)
                    nc.vector.tensor_mul(t2[:, :, 64:128], q3[:, :, 0:64], sinb)
                    qrot = rsb.tile([128, QN], BF16, tag="qrot")
                    nc.vector.tensor_add(
                        qrot, t1.rearrange("p g x -> p (g x)"),
                        t2.rearrange("p g x -> p (g x)"))

                    # --- RoPE on k (fp32 result kept for the cache output) ---
                    cos1 = rope_tab(cos_sb, tt, 1)
                    sin1 = rope_tab(sin_sb, tt, 1)
                    t1k = rsb.tile([128, H], F32, tag="t1k")
                    t2k = rsb.tile([128, H], F32, tag="t2k")
                    nc.vector.tensor_mul(t1k[:, 0:64], ps_kv[:, 0:64], cos1)
                    nc.vector.tensor_mul(t1k[:, 64:128], ps_kv[:, 64:128], cos1)
                    nc.vector.scalar_tensor_tensor(
                        out=t2k[:, 0:64], in0=ps_kv[:, 64:128], scalar=-1.0,
                        in1=sin1, op0=OP.mult, op1=OP.mult)
                    nc.vector.tensor_mul(t2k[:, 64:128], ps_kv[:, 0:64], sin1)
                    kf = ssb.tile([128, H], F32, tag="kf")
                    nc.vector.tensor_add(kf, t1k, t2k)
                    nc.sync.dma_start(
                        out=k_out.ap()[tt * 128:(tt + 1) * 128, :], in_=kf)
                    kb = ssb.tile([128, H], BF16, tag="kb")
                    nc.vector.tensor_copy(kb, kf)

                    # --- v: copy out fp32 + keep bf16 for PV ---
                    vf = ssb.tile([128, H], F32, tag="vf")
                    nc.vector.tensor_copy(vf, ps_kv[:, H:2 * H])
                    nc.sync.dma_start(
                        out=v_out.ap()[tt * 128:(tt + 1) * 128, :], in_=vf)
                    nc.vector.tensor_copy(vb_sb[:, tt, :], ps_kv[:, H:2 * H])

                    # --- transposes into [h, t] attention layout, via the
                    # scalar HWDGE queue's xbar (keeps PE + DVE free) ---
                    for g in range(NH):
                        nc.scalar.dma_start_transpose(
                            out=qT_sb[:, g, tt * 128:(tt + 1) * 128],
                            in_=qrot[:, g * H:(g + 1) * H])
                    nc.scalar.dma_start_transpose(
                        out=kT_sb[:, tt * 128:(tt + 1) * 128], in_=kb)

                def proj_mms(x_sb, ps_q, ps_kv, kk0, kk1):
                    for kk in range(kk0, kk1):
                        st = kk == 0
                        sp = kk == cfg.n_dtile - 1
                        nc.tensor.matmul(ps_q, x_sb[:, kk, :], wq_sb[:, kk, :],
                                         start=st, stop=sp)
                        nc.tensor.matmul(ps_kv, x_sb[:, kk, :], wkv_sb[:, kk, :],
                                         start=st, stop=sp)

                for tt in range(cfg.n_ttile):
                    x_sb = x_pre if tt == 0 else load_x(tt)
                    ps_q = pps.tile([128, QN], F32, tag="ps_q")
                    ps_kv = pps.tile([128, 2 * H], F32, tag="ps_kv")
                    proj_mms(x_sb, ps_q, ps_kv, 0, cfg.n_dtile)
                    proj_epilogue(tt, ps_q, ps_kv)

            # ---------------- attention + O projection, per sequence --------
            wo_sb = wpool.tile([128, NH, cfg.n_dtile, 128], BF16, tag="wo")
            nc.sync.dma_start(out=wo_sb, in_=wo_d.ap())

            scale = float(H) ** -0.5
            with tc.tile_pool(name="attn_sb", bufs=8) as asb, \
                 tc.tile_pool(name="outT_sb", bufs=2) as osb, \
                 tc.tile_pool(name="norm_sb", bufs=3) as nsb, \
                 tc.tile_pool(name="ostage_sb", bufs=6) as ostg, \
                 tc.tile_pool(name="sT_ps", bufs=3, space="PSUM") as sps, \
                 tc.tile_pool(name="outT_ps", bufs=2, space="PSUM") as ops, \
                 tc.tile_pool(name="z_ps", bufs=1, space="PSUM") as zps, \
                 tc.tile_pool(name="o_ps", bufs=2, space="PSUM") as oops:
                GW = NH * 128  # all heads side by side in the moving dim
                tq_per_chunk = CHUNK // 128

                pending = []  # (s, jc, dt0, dt1, outT) o-proj slices to weave

                def oproj_slice(s, jc, dt0, dt1, outT):
                    # O projection slice (wo stationary -> oT output [D, T];
                    # host transposes once at the end). Slices are woven
                    # between q-tiles as PE filler while ACT runs exp.
                    for dt in range(dt0, dt1):
                        ps_o = oops.tile([128, CHUNK], F32, tag="pso",
                                         name=f"pso_{s}_{jc}_{dt}")
                        for g in range(NH):
                            nc.tensor.matmul(
                                ps_o, wo_sb[:, g, dt, :],
                                outT[:, g, jc * CHUNK:(jc + 1) * CHUNK],
                                start=(g == 0), stop=(g == NH - 1))
                        o_sb = ostg.tile([128, CHUNK], BF16, tag="osb")
                        nc.vector.tensor_copy(o_sb, ps_o)
                        col0 = s * SQ + jc * CHUNK
                        nc.sync.dma_start(
                            out=o_out.ap()[dt * 128:(dt + 1) * 128,
                                           col0:col0 + CHUNK],
                            in_=o_sb)

                n_slice = max(cfg.n_dtile // 4, 1)  # dt per woven slice

                for s in range(cfg.n_seq):
                    outT = osb.tile([128, NH, SQ], BF16, tag="outT")
                    for tq in range(cfg.tt_per_seq):
                        if pending:
                            oproj_slice(*pending.pop(0))
                        q0 = s * SQ + tq * 128
                        # moving operand: 128 q-cols of every head (strided)
                        qmov = qT_sb[:, :, q0:q0 + 128]
                        ps_out = ops.tile([128, GW], F32, tag="po")
                        ps_z = zps.tile([128, GW], F32, tag="pz")
                        for i in range(tq + 1):
                            st_g = s * cfg.tt_per_seq + i
                            ps_s = sps.tile([128, GW], F32, tag="ps")
                            nc.tensor.matmul(
                                ps_s.rearrange("p (g x) -> p g x", g=NH),
                                kT_sb[:, st_g * 128:(st_g + 1) * 128],
                                qmov, start=True, stop=True)
                            pt = asb.tile([128, GW], BF16, tag="pt")
                            nc.scalar.activation(pt, ps_s, AF.Exp, scale=scale)
                            if i == tq:  # diagonal: zero the causal upper part
                                ptm = asb.tile([128, GW], BF16, tag="pt")
                                nc.vector.tensor_mul(ptm, pt, mask_sb.ap())
                                pt = ptm
                            nc.tensor.matmul(
                                ps_out, vb_sb[:, st_g, :], pt,
                                start=(i == 0), stop=(i == tq))
                            nc.tensor.matmul(
                                ps_z, ones_sb.ap(), pt,
                                start=(i == 0), stop=(i == tq))
                        rz = nsb.tile([128, GW], F32, tag="rz")
                        nc.vector.reciprocal_approx_fast(out=rz, in_=ps_z)
                        nc.vector.tensor_mul(
                            outT[:, :, tq * 128:(tq + 1) * 128],
                            ps_out.rearrange("p (g x) -> p g x", g=NH),
                            rz.rearrange("p (g x) -> p g x", g=NH))
                        if (tq + 1) % tq_per_chunk == 0:
                            jc = tq // tq_per_chunk
                            for dt0 in range(0, cfg.n_dtile, n_slice):
                                pending.append(
                                    (s, jc, dt0, dt0 + n_slice, outT))
                for item in pending:  # drain the tail (last chunks)
                    oproj_slice(*item)

    nc.compile()
    return nc


# ---------------------------------------------------------------------------
# host side
# ---------------------------------------------------------------------------

def make_in_maps(cfg: Cfg, x, w_q, w_k, w_v, w_o, positions, n_cores=NCORES):
    """Per-core input maps. x:[T,D] f32, w_q:[D,N,H], w_k/w_v:[D,K,H],
    w_o:[N,H,D], positions:[T] int32."""
    bf16 = ml_dtypes.bfloat16
    D, T, NH, H = cfg.D, cfg.T, cfg.NH, cfg.H
    nt, nd, DC = cfg.n_ttile, cfg.n_dtile, cfg.DC
    # x swizzled to the exact SBUF layout: [tt, p(=d within tile), kk, t]
    xt = np.ascontiguousarray(
        x.reshape(nt, 128, nd, 128).transpose(0, 3, 2, 1)
         .reshape(nt, 128, nd * 128)).astype(bf16)
    half = H // 2
    inv_freq = (10000.0 ** (-np.arange(half, dtype=np.float64) * 2.0 / H))
    ang = positions.astype(np.float64)[:, None] * inv_freq[None, :]
    cosm = np.ascontiguousarray(
        np.cos(ang).astype(np.float32).reshape(nt, 128, 64)
        .transpose(1, 0, 2).reshape(128, nt * 64))
    sinm = np.ascontiguousarray(
        np.sin(ang).astype(np.float32).reshape(nt, 128, 64)
        .transpose(1, 0, 2).reshape(128, nt * 64))
    maskS = np.tile(
        np.where(np.arange(128)[:, None] <= np.arange(128)[None, :],
                 1.0, 0.0), (1, NH)).astype(bf16)

    def sw_w(w):  # [D, n] -> [p, kk, n] -> [128, kk*n]
        n = w.shape[1]
        return np.ascontiguousarray(
            w.reshape(nd, 128, n).transpose(1, 0, 2).reshape(128, nd * n))

    in_maps = []
    for c in range(n_cores):
        wq_c = sw_w(w_q[:, c * NH:(c + 1) * NH, :].reshape(D, NH * H)).astype(bf16)
        wkv_c = sw_w(np.concatenate(
            [w_k[:, c, :], w_v[:, c, :]], axis=1)).astype(bf16)
        wo_c = np.ascontiguousarray(
            w_o[c * NH:(c + 1) * NH].reshape(NH, H, nd, 128)
            .transpose(1, 0, 2, 3).reshape(128, NH * nd * 128)).astype(bf16)
        in_maps.append(dict(xt=xt, wq=wq_c, wkv=wkv_c, wo=wo_c,
                            cosm=cosm, sinm=sinm, maskS=maskS))
    return in_maps


_NC_CACHE = {}
_RUN_EXTRA = {}   # test harness can set e.g. {"trace": True}
_LAST_RES = {}    # test harness reads back the BassKernelResults


def kernel(x, k_cache, v_cache, w_q, w_k, w_v, w_o,
           block_tables, positions, slot_mapping, seq_lens):
    x = np.asarray(x); w_q = np.asarray(w_q); w_k = np.asarray(w_k)
    w_v = np.asarray(w_v); w_o = np.asarray(w_o)
    positions = np.asarray(positions); slot_mapping = np.asarray(slot_mapping)
    k_cache = np.asarray(k_cache); v_cache = np.asarray(v_cache)

    cfg = Cfg()
    T, D = x.shape
    K = w_k.shape[1]
    assert (T, D) == (cfg.T, cfg.D) and K == NCORES

    if "nc" not in _NC_CACHE:
        _NC_CACHE["nc"] = build_kernel(cfg)
    nc = _NC_CACHE["nc"]

    in_maps = make_in_maps(cfg, x, w_q, w_k, w_v, w_o, positions)
    res = bass_utils.run_bass_kernel_spmd(
        nc, in_maps, core_ids=list(range(NCORES)), **_RUN_EXTRA)
    _LAST_RES["res"] = res

    H = cfg.H
    ot = np.zeros((D, T), np.float32)
    k_full = np.empty((T, K, H), np.float32)
    v_full = np.empty((T, K, H), np.float32)
    for c in range(NCORES):
        r = res.results[c]
        ot += r["ot_part"].astype(np.float32)
        k_full[:, c, :] = r["k_out"]
        v_full[:, c, :] = r["v_out"]
    o = np.ascontiguousarray(ot.T)

    kc = k_cache.reshape(-1, K, H).copy()
    vc = v_cache.reshape(-1, K, H).copy()
    kc[slot_mapping] = k_full
    vc[slot_mapping] = v_full
    return (kc.reshape(k_cache.shape).astype(np.float32),
            vc.reshape(v_cache.shape).astype(np.float32),
            o)


# revision 42
# speedup vs baseline: 1.3375x; 1.3375x over previous
"""Paged-attention prefill kernel for Trainium2, tensor-parallel over heads on 8 cores.

Reference semantics (see problem): full prefill GQA attention with RoPE and a
paged KV cache. B=4 seqs x 1024 tokens, D=4096, 32 q heads / 8 kv heads, H=128.

Sharding: core c owns q heads 4c..4c+3 and kv head c. x is replicated (passed
pre-transposed as xT), w_q/w_o sharded on N, w_k/w_v on K. Each core computes
its 4 heads' attention and a partial O projection [T, D]; the host sums the 8
partials. k/v (post-RoPE) are returned per-core in token order; the host
scatters them into the paged caches via slot_mapping.

Device layout choices:
  - projections produce q/k/v in [t, h] layout (moving operand = weights,
    N=512/256), RoPE applied with free-dim strided views (no partition moves),
    then PE-transposes give qT/kT in [h, t] for attention.
  - scores are computed transposed: per 128-token q-tile, ONE matmul produces
    sT [s-tile, 4 heads x 128 q] with the 4 heads side by side in the moving
    dim (strided view of qT), so each LDWEIGHTS serves 4 heads' work. exp
    needs no max subtraction (scores ~ N(0,1), no overflow); causality: s-tiles
    above the diagonal are skipped, the diagonal tile is masked by a 0/1
    multiply on DVE after exp.
  - softmax denominator Z = ones[128,128]^T @ p accumulated in PSUM (M=128 so
    it pipelines like a normal matmul; rows are replicated so no partition
    broadcast is needed), then outT *= reciprocal_approx_fast(Z).
  - O projection keeps w_o stationary (reused across both 512-token moving
    chunks) and emits oT [D, T] partials in bf16; the host sums 8 partials in
    fp32 and transposes once.
  - all inputs are host-swizzled into the exact SBUF layouts so every DMA is
    partition-contiguous; leading x/weight DMAs are split fine-grained and
    spread across HWDGE queues so the first matmul starts at ~10us.
Compute dtype bf16 (fp32 PSUM accumulation).
"""
import numpy as np
import ml_dtypes

import concourse.bass as bass
import concourse.bacc as bacc
import concourse.mybir as mybir
import concourse.tile as tile
from concourse import bass_utils
from concourse.masks import make_identity

F32 = mybir.dt.float32
BF16 = mybir.dt.bfloat16
AF = mybir.ActivationFunctionType
OP = mybir.AluOpType

NCORES = 8


class Cfg:
    def __init__(self, D=4096, T=4096, SQ=1024, NH=4, CHUNK=512):
        self.D, self.T, self.SQ, self.NH, self.CHUNK = D, T, SQ, NH, CHUNK
        self.H = 128
        self.n_dtile = D // 128
        self.n_ttile = T // 128
        self.n_seq = T // SQ
        self.tt_per_seq = SQ // 128
        self.st_per_chunk = CHUNK // 128
        self.chunks_per_seq = SQ // CHUNK
        self.DC = D // 512  # d-chunks for O projection
        assert SQ % CHUNK == 0 and CHUNK % 128 == 0 and D % 512 == 0


def build_kernel(cfg: Cfg):
    D, T, SQ, NH, CHUNK = cfg.D, cfg.T, cfg.SQ, cfg.NH, cfg.CHUNK
    H = cfg.H
    QN = NH * H  # q projection width per core (4 heads -> 512)

    nc = bacc.Bacc("TRN2", target_bir_lowering=False, debug=False)

    # ---- DRAM I/O (per-core), all pre-swizzled on host for contiguous DMA ----
    x_d = nc.dram_tensor("xt", [cfg.n_ttile, 128, cfg.n_dtile * 128], BF16,
                         kind="ExternalInput")
    wq_d = nc.dram_tensor("wq", [128, cfg.n_dtile * QN], BF16,
                          kind="ExternalInput")
    wkv_d = nc.dram_tensor("wkv", [128, cfg.n_dtile * 2 * H], BF16,
                           kind="ExternalInput")
    wo_d = nc.dram_tensor("wo", [128, NH * cfg.DC * 512], BF16,
                          kind="ExternalInput")
    cos_d = nc.dram_tensor("cosm", [128, cfg.n_ttile * 64], F32,
                           kind="ExternalInput")
    sin_d = nc.dram_tensor("sinm", [128, cfg.n_ttile * 64], F32,
                           kind="ExternalInput")
    mask_d = nc.dram_tensor("maskS", [128, NH * 128], BF16,
                            kind="ExternalInput")

    k_out = nc.dram_tensor("k_out", [T, H], F32, kind="ExternalOutput")
    v_out = nc.dram_tensor("v_out", [T, H], F32, kind="ExternalOutput")
    o_out = nc.dram_tensor("ot_part", [D, T], BF16, kind="ExternalOutput")

    with tile.TileContext(nc) as tc:
        # persistent constants in raw SBUF (concrete addresses -> manual
        # broadcast APs are safe)
        cos_sb = nc.alloc_sbuf_tensor("cos_sb", [128, cfg.n_ttile * 64], F32)
        sin_sb = nc.alloc_sbuf_tensor("sin_sb", [128, cfg.n_ttile * 64], F32)

        def rope_tab(sb, tt, nbc):
            """[128, nbc(bcast), 64] view of table column block tt."""
            ap = sb.ap()
            if nbc == 1:
                return ap[:, tt * 64:(tt + 1) * 64]
            return bass.AP(tensor=ap.tensor, offset=ap.offset + tt * 64,
                           ap=[list(ap.ap[0]), [0, nbc], [1, 64]])

        mask_sb = nc.alloc_sbuf_tensor("mask_sb", [128, NH * 128], BF16)
        ident = nc.alloc_sbuf_tensor("ident_sb", [128, 128], BF16)
        ones_sb = nc.alloc_sbuf_tensor("ones_sb", [128, 128], BF16)

        with tc.tile_pool(name="persist", bufs=1) as persist, \
             tc.tile_pool(name="weights", bufs=1) as wpool:
            # resident weights
            wq_sb = wpool.tile([128, cfg.n_dtile, QN], BF16, tag="wq")
            wkv_sb = wpool.tile([128, cfg.n_dtile, 2 * H], BF16, tag="wkv")

            # attention operand stores (full T)
            qT_sb = persist.tile([128, NH, T], BF16, tag="qT")
            kT_sb = persist.tile([128, T], BF16, tag="kT")
            vb_sb = persist.tile([128, cfg.n_ttile, H], BF16, tag="vb")

            # ---------------- projection + RoPE + transposes ----------------
            with tc.tile_pool(name="proj_sb", bufs=4) as psb, \
                 tc.tile_pool(name="rope_sb", bufs=3) as rsb, \
                 tc.tile_pool(name="stage_sb", bufs=3) as ssb, \
                 tc.tile_pool(name="proj_ps", bufs=2, space="PSUM") as pps, \
                 tc.tile_pool(name="tr_ps", bufs=2, space="PSUM") as tps:
                def load_x(tt, steps=None, eng=None):
                    x_sb = psb.tile([128, cfg.n_dtile, 128], BF16, tag="x")
                    base = max(cfg.n_dtile // 4, 1)
                    sched = list(steps or [])
                    q4 = 0
                    while q4 < cfg.n_dtile:
                        step = min(sched.pop(0) if sched else base,
                                   cfg.n_dtile - q4)
                        (eng or nc.sync).dma_start(
                            out=x_sb[:, q4:q4 + step, :],
                            in_=x_d.ap()[tt, :, q4 * 128:(q4 + step) * 128])
                        q4 += step
                    return x_sb

                # startup order: x0, then weights chunk-interleaved, then consts
                x_pre = load_x(0, steps=[2, 2, 4], eng=nc.scalar)
                wsched = [1, 1, 2] if cfg.n_dtile >= 8 else []
                kq = 0
                while kq < cfg.n_dtile:
                    step = min(wsched.pop(0) if wsched else 4,
                               cfg.n_dtile - kq)
                    nc.sync.dma_start(
                        out=wq_sb[:, kq:kq + step, :],
                        in_=wq_d.ap()[:, kq * QN:(kq + step) * QN])
                    nc.sync.dma_start(
                        out=wkv_sb[:, kq:kq + step, :],
                        in_=wkv_d.ap()[:, kq * 2 * H:(kq + step) * 2 * H])
                    kq += step
                nc.scalar.dma_start(out=cos_sb.ap(), in_=cos_d.ap())
                nc.scalar.dma_start(out=sin_sb.ap(), in_=sin_d.ap())
                nc.scalar.dma_start(out=mask_sb.ap(), in_=mask_d.ap())
                make_identity(nc, ident.ap())
                nc.vector.memset(ones_sb.ap(), 1.0)

                def proj_epilogue(tt, ps_q, ps_kv):
                    # --- RoPE on q (all NH heads at once via strided views) ---
                    q3 = ps_q.rearrange("p (g x) -> p g x", g=NH)
                    cosb = rope_tab(cos_sb, tt, NH)
                    sinb = rope_tab(sin_sb, tt, NH)
                    t1 = rsb.tile([128, NH, H], F32, tag="t1")
                    t2 = rsb.tile([128, NH, H], F32, tag="t2")
                    nc.vector.tensor_mul(t1[:, :, 0:64], q3[:, :, 0:64], cosb)
                    nc.vector.tensor_mul(t1[:, :, 64:128], q3[:, :, 64:128], cosb)
                    nc.vector.scalar_tensor_tensor(
                        out=t2[:, :, 0:64], in0=q3[:, :, 64:128], scalar=-1.0,
                        in1=sinb, op0=OP.mult, op1=OP.mult)
                    nc.vector.tensor_mul(t2[:, :, 64:128], q3[:, :, 0:64], sinb)
                    qrot = rsb.tile([128, QN], BF16, tag="qrot")
                    nc.vector.tensor_add(
                        qrot, t1.rearrange("p g x -> p (g x)"),
                        t2.rearrange("p g x -> p (g x)"))

                    # --- RoPE on k (fp32 result kept for the cache output) ---
                    cos1 = rope_tab(cos_sb, tt, 1)
                    sin1 = rope_tab(sin_sb, tt, 1)
                    t1k = rsb.tile([128, H], F32, tag="t1k")
                    t2k = rsb.tile([128, H], F32, tag="t2k")
                    nc.vector.tensor_mul(t1k[:, 0:64], ps_kv[:, 0:64], cos1)
                    nc.vector.tensor_mul(t1k[:, 64:128], ps_kv[:, 64:128], cos1)
                    nc.vector.scalar_tensor_tensor(
                        out=t2k[:, 0:64], in0=ps_kv[:, 64:128], scalar=-1.0,
                        in1=sin1, op0=OP.mult, op1=OP.mult)
                    nc.vector.tensor_mul(t2k[:, 64:128], ps_kv[:, 0:64], sin1)
                    kf = ssb.tile([128, H], F32, tag="kf")
                    nc.vector.tensor_add(kf, t1k, t2k)
                    nc.sync.dma_start(
                        out=k_out.ap()[tt * 128:(tt + 1) * 128, :], in_=kf)
                    kb = ssb.tile([128, H], BF16, tag="kb")
                    nc.vector.tensor_copy(kb, kf)

                    # --- v: copy out fp32 + keep bf16 for PV ---
                    vf = ssb.tile([128, H], F32, tag="vf")
                    nc.vector.tensor_copy(vf, ps_kv[:, H:2 * H])
                    nc.sync.dma_start(
                        out=v_out.ap()[tt * 128:(tt + 1) * 128, :], in_=vf)
                    nc.vector.tensor_copy(vb_sb[:, tt, :], ps_kv[:, H:2 * H])

                    # --- transposes into [h, t] attention layout ---
                    for g in range(NH):
                        ptr = tps.tile([128, 128], BF16, tag="ptr")
                        nc.tensor.transpose(
                            ptr, qrot[:, g * H:(g + 1) * H], ident.ap())
                        nc.vector.tensor_copy(
                            qT_sb[:, g, tt * 128:(tt + 1) * 128], ptr)
                    ptrk = tps.tile([128, 128], BF16, tag="ptr")
                    nc.tensor.transpose(ptrk, kb, ident.ap())
                    nc.vector.tensor_copy(kT_sb[:, tt * 128:(tt + 1) * 128], ptrk)

                def proj_mms(x_sb, ps_q, ps_kv, kk0, kk1):
                    for kk in range(kk0, kk1):
                        st = kk == 0
                        sp = kk == cfg.n_dtile - 1
                        nc.tensor.matmul(ps_q, x_sb[:, kk, :], wq_sb[:, kk, :],
                                         start=st, stop=sp)
                        nc.tensor.matmul(ps_kv, x_sb[:, kk, :], wkv_sb[:, kk, :],
                                         start=st, stop=sp)

                for tt in range(cfg.n_ttile):
                    x_sb = x_pre if tt == 0 else load_x(tt)
                    ps_q = pps.tile([128, QN], F32, tag="ps_q")
                    ps_kv = pps.tile([128, 2 * H], F32, tag="ps_kv")
                    proj_mms(x_sb, ps_q, ps_kv, 0, cfg.n_dtile)
                    proj_epilogue(tt, ps_q, ps_kv)

            # ---------------- attention + O projection, per sequence --------
            wo_sb = wpool.tile([128, NH, cfg.n_dtile, 128], BF16, tag="wo")
            nc.sync.dma_start(out=wo_sb, in_=wo_d.ap())

            scale = float(H) ** -0.5
            with tc.tile_pool(name="attn_sb", bufs=8) as asb, \
                 tc.tile_pool(name="outT_sb", bufs=2) as osb, \
                 tc.tile_pool(name="norm_sb", bufs=3) as nsb, \
                 tc.tile_pool(name="ostage_sb", bufs=6) as ostg, \
                 tc.tile_pool(name="sT_ps", bufs=3, space="PSUM") as sps, \
                 tc.tile_pool(name="outT_ps", bufs=2, space="PSUM") as ops, \
                 tc.tile_pool(name="z_ps", bufs=1, space="PSUM") as zps, \
                 tc.tile_pool(name="o_ps", bufs=2, space="PSUM") as oops:
                GW = NH * 128  # all heads side by side in the moving dim
                tq_per_chunk = CHUNK // 128

                pending = []  # (s, jc, dt0, dt1, outT) o-proj slices to weave

                def oproj_slice(s, jc, dt0, dt1, outT):
                    # O projection slice (wo stationary -> oT output [D, T];
                    # host transposes once at the end). Slices are woven
                    # between q-tiles as PE filler while ACT runs exp.
                    for dt in range(dt0, dt1):
                        ps_o = oops.tile([128, CHUNK], F32, tag="pso",
                                         name=f"pso_{s}_{jc}_{dt}")
                        for g in range(NH):
                            nc.tensor.matmul(
                                ps_o, wo_sb[:, g, dt, :],
                                outT[:, g, jc * CHUNK:(jc + 1) * CHUNK],
                                start=(g == 0), stop=(g == NH - 1))
                        o_sb = ostg.tile([128, CHUNK], BF16, tag="osb")
                        nc.vector.tensor_copy(o_sb, ps_o)
                        col0 = s * SQ + jc * CHUNK
                        nc.sync.dma_start(
                            out=o_out.ap()[dt * 128:(dt + 1) * 128,
                                           col0:col0 + CHUNK],
                            in_=o_sb)

                n_slice = max(cfg.n_dtile // 4, 1)  # dt per woven slice

                for s in range(cfg.n_seq):
                    outT = osb.tile([128, NH, SQ], BF16, tag="outT")
                    for tq in range(cfg.tt_per_seq):
                        if pending:
                            oproj_slice(*pending.pop(0))
                        q0 = s * SQ + tq * 128
                        # moving operand: 128 q-cols of every head (strided)
                        qmov = qT_sb[:, :, q0:q0 + 128]
                        ps_out = ops.tile([128, GW], F32, tag="po")
                        ps_z = zps.tile([128, GW], F32, tag="pz")
                        for i in range(tq + 1):
                            st_g = s * cfg.tt_per_seq + i
                            ps_s = sps.tile([128, GW], F32, tag="ps")
                            nc.tensor.matmul(
                                ps_s.rearrange("p (g x) -> p g x", g=NH),
                                kT_sb[:, st_g * 128:(st_g + 1) * 128],
                                qmov, start=True, stop=True)
                            pt = asb.tile([128, GW], BF16, tag="pt")
                            nc.scalar.activation(pt, ps_s, AF.Exp, scale=scale)
                            if i == tq:  # diagonal: zero the causal upper part
                                ptm = asb.tile([128, GW], BF16, tag="pt")
                                nc.vector.tensor_mul(ptm, pt, mask_sb.ap())
                                pt = ptm
                            nc.tensor.matmul(
                                ps_out, vb_sb[:, st_g, :], pt,
                                start=(i == 0), stop=(i == tq))
                            nc.tensor.matmul(
                                ps_z, ones_sb.ap(), pt,
                                start=(i == 0), stop=(i == tq))
                        rz = nsb.tile([128, GW], F32, tag="rz")
                        nc.vector.reciprocal_approx_fast(out=rz, in_=ps_z)
                        nc.vector.tensor_mul(
                            outT[:, :, tq * 128:(tq + 1) * 128],
                            ps_out.rearrange("p (g x) -> p g x", g=NH),
                            rz.rearrange("p (g x) -> p g x", g=NH))
                        if (tq + 1) % tq_per_chunk == 0:
                            jc = tq // tq_per_chunk
                            for dt0 in range(0, cfg.n_dtile, n_slice):
                                pending.append(
                                    (s, jc, dt0, dt0 + n_slice, outT))
                for item in pending:  # drain the tail (last chunks)
                    oproj_slice(*item)

    nc.compile()
    return nc


# ---------------------------------------------------------------------------
# host side
# ---------------------------------------------------------------------------

def make_in_maps(cfg: Cfg, x, w_q, w_k, w_v, w_o, positions, n_cores=NCORES):
    """Per-core input maps. x:[T,D] f32, w_q:[D,N,H], w_k/w_v:[D,K,H],
    w_o:[N,H,D], positions:[T] int32."""
    bf16 = ml_dtypes.bfloat16
    D, T, NH, H = cfg.D, cfg.T, cfg.NH, cfg.H
    nt, nd, DC = cfg.n_ttile, cfg.n_dtile, cfg.DC
    # x swizzled to the exact SBUF layout: [tt, p(=d within tile), kk, t]
    xt = np.ascontiguousarray(
        x.reshape(nt, 128, nd, 128).transpose(0, 3, 2, 1)
         .reshape(nt, 128, nd * 128)).astype(bf16)
    half = H // 2
    inv_freq = (10000.0 ** (-np.arange(half, dtype=np.float64) * 2.0 / H))
    ang = positions.astype(np.float64)[:, None] * inv_freq[None, :]
    cosm = np.ascontiguousarray(
        np.cos(ang).astype(np.float32).reshape(nt, 128, 64)
        .transpose(1, 0, 2).reshape(128, nt * 64))
    sinm = np.ascontiguousarray(
        np.sin(ang).astype(np.float32).reshape(nt, 128, 64)
        .transpose(1, 0, 2).reshape(128, nt * 64))
    maskS = np.tile(
        np.where(np.arange(128)[:, None] <= np.arange(128)[None, :],
                 1.0, 0.0), (1, NH)).astype(bf16)

    def sw_w(w):  # [D, n] -> [p, kk, n] -> [128, kk*n]
        n = w.shape[1]
        return np.ascontiguousarray(
            w.reshape(nd, 128, n).transpose(1, 0, 2).reshape(128, nd * n))

    in_maps = []
    for c in range(n_cores):
        wq_c = sw_w(w_q[:, c * NH:(c + 1) * NH, :].reshape(D, NH * H)).astype(bf16)
        wkv_c = sw_w(np.concatenate(
            [w_k[:, c, :], w_v[:, c, :]], axis=1)).astype(bf16)
        wo_c = np.ascontiguousarray(
            w_o[c * NH:(c + 1) * NH].reshape(NH, H, nd, 128)
            .transpose(1, 0, 2, 3).reshape(128, NH * nd * 128)).astype(bf16)
        in_maps.append(dict(xt=xt, wq=wq_c, wkv=wkv_c, wo=wo_c,
                            cosm=cosm, sinm=sinm, maskS=maskS))
    return in_maps


_NC_CACHE = {}
_RUN_EXTRA = {}   # test harness can set e.g. {"trace": True}
_LAST_RES = {}    # test harness reads back the BassKernelResults


def kernel(x, k_cache, v_cache, w_q, w_k, w_v, w_o,
           block_tables, positions, slot_mapping, seq_lens):
    x = np.asarray(x); w_q = np.asarray(w_q); w_k = np.asarray(w_k)
    w_v = np.asarray(w_v); w_o = np.asarray(w_o)
    positions = np.asarray(positions); slot_mapping = np.asarray(slot_mapping)
    k_cache = np.asarray(k_cache); v_cache = np.asarray(v_cache)

    cfg = Cfg()
    T, D = x.shape
    K = w_k.shape[1]
    assert (T, D) == (cfg.T, cfg.D) and K == NCORES

    if "nc" not in _NC_CACHE:
        _NC_CACHE["nc"] = build_kernel(cfg)
    nc = _NC_CACHE["nc"]

    in_maps = make_in_maps(cfg, x, w_q, w_k, w_v, w_o, positions)
    res = bass_utils.run_bass_kernel_spmd(
        nc, in_maps, core_ids=list(range(NCORES)), **_RUN_EXTRA)
    _LAST_RES["res"] = res

    H = cfg.H
    ot = np.zeros((D, T), np.float32)
    k_full = np.empty((T, K, H), np.float32)
    v_full = np.empty((T, K, H), np.float32)
    for c in range(NCORES):
        r = res.results[c]
        ot += r["ot_part"].astype(np.float32)
        k_full[:, c, :] = r["k_out"]
        v_full[:, c, :] = r["v_out"]
    o = np.ascontiguousarray(ot.T)

    kc = k_cache.reshape(-1, K, H).copy()
    vc = v_cache.reshape(-1, K, H).copy()
    kc[slot_mapping] = k_full
    vc[slot_mapping] = v_full
    return (kc.reshape(k_cache.shape).astype(np.float32),
            vc.reshape(v_cache.shape).astype(np.float32),
            o)


# revision 43
# speedup vs baseline: 1.3612x; 1.0177x over previous
"""Paged-attention prefill kernel for Trainium2, tensor-parallel over heads on 8 cores.

Reference semantics (see problem): full prefill GQA attention with RoPE and a
paged KV cache. B=4 seqs x 1024 tokens, D=4096, 32 q heads / 8 kv heads, H=128.

Sharding: core c owns q heads 4c..4c+3 and kv head c. x is replicated (passed
pre-transposed as xT), w_q/w_o sharded on N, w_k/w_v on K. Each core computes
its 4 heads' attention and a partial O projection [T, D]; the host sums the 8
partials. k/v (post-RoPE) are returned per-core in token order; the host
scatters them into the paged caches via slot_mapping.

Device layout choices:
  - projections produce q/k/v in [t, h] layout (moving operand = weights,
    N=512/256), RoPE applied with free-dim strided views (no partition moves),
    then PE-transposes give qT/kT in [h, t] for attention.
  - scores are computed transposed: per 128-token q-tile, ONE matmul produces
    sT [s-tile, 4 heads x 128 q] with the 4 heads side by side in the moving
    dim (strided view of qT), so each LDWEIGHTS serves 4 heads' work. exp
    needs no max subtraction (scores ~ N(0,1), no overflow); causality: s-tiles
    above the diagonal are skipped, the diagonal tile is masked by a 0/1
    multiply on DVE after exp.
  - softmax denominator Z = ones[128,128]^T @ p accumulated in PSUM (M=128 so
    it pipelines like a normal matmul; rows are replicated so no partition
    broadcast is needed), then outT *= reciprocal_approx_fast(Z).
  - O projection keeps w_o stationary (reused across both 512-token moving
    chunks) and emits oT [D, T] partials in bf16; the host sums 8 partials in
    fp32 and transposes once.
  - all inputs are host-swizzled into the exact SBUF layouts so every DMA is
    partition-contiguous; leading x/weight DMAs are split fine-grained and
    spread across HWDGE queues so the first matmul starts at ~10us.
Compute dtype bf16 (fp32 PSUM accumulation).
"""
import numpy as np
import ml_dtypes

import concourse.bass as bass
import concourse.bacc as bacc
import concourse.mybir as mybir
import concourse.tile as tile
from concourse import bass_utils
from concourse.masks import make_identity

F32 = mybir.dt.float32
BF16 = mybir.dt.bfloat16
AF = mybir.ActivationFunctionType
OP = mybir.AluOpType

NCORES = 8


class Cfg:
    def __init__(self, D=4096, T=4096, SQ=1024, NH=4, CHUNK=512):
        self.D, self.T, self.SQ, self.NH, self.CHUNK = D, T, SQ, NH, CHUNK
        self.H = 128
        self.n_dtile = D // 128
        self.n_ttile = T // 128
        self.n_seq = T // SQ
        self.tt_per_seq = SQ // 128
        self.st_per_chunk = CHUNK // 128
        self.chunks_per_seq = SQ // CHUNK
        self.DC = D // 512  # d-chunks for O projection
        assert SQ % CHUNK == 0 and CHUNK % 128 == 0 and D % 512 == 0


def build_kernel(cfg: Cfg):
    D, T, SQ, NH, CHUNK = cfg.D, cfg.T, cfg.SQ, cfg.NH, cfg.CHUNK
    H = cfg.H
    QN = NH * H  # q projection width per core (4 heads -> 512)

    nc = bacc.Bacc("TRN2", target_bir_lowering=False, debug=False)

    # ---- DRAM I/O (per-core), all pre-swizzled on host for contiguous DMA ----
    x_d = nc.dram_tensor("xt", [cfg.n_ttile, 128, cfg.n_dtile * 128], BF16,
                         kind="ExternalInput")
    wq_d = nc.dram_tensor("wq", [128, cfg.n_dtile * QN], BF16,
                          kind="ExternalInput")
    wkv_d = nc.dram_tensor("wkv", [128, cfg.n_dtile * 2 * H], BF16,
                           kind="ExternalInput")
    wo_d = nc.dram_tensor("wo", [128, NH * cfg.DC * 512], BF16,
                          kind="ExternalInput")
    cos_d = nc.dram_tensor("cosm", [128, cfg.n_ttile * 64], F32,
                           kind="ExternalInput")
    sin_d = nc.dram_tensor("sinm", [128, cfg.n_ttile * 64], F32,
                           kind="ExternalInput")
    mask_d = nc.dram_tensor("maskS", [128, NH * 128], BF16,
                            kind="ExternalInput")

    k_out = nc.dram_tensor("k_out", [T, H], F32, kind="ExternalOutput")
    v_out = nc.dram_tensor("v_out", [T, H], F32, kind="ExternalOutput")
    o_out = nc.dram_tensor("ot_part", [D, T], BF16, kind="ExternalOutput")

    with tile.TileContext(nc) as tc:
        # persistent constants in raw SBUF (concrete addresses -> manual
        # broadcast APs are safe)
        cos_sb = nc.alloc_sbuf_tensor("cos_sb", [128, cfg.n_ttile * 64], F32)
        sin_sb = nc.alloc_sbuf_tensor("sin_sb", [128, cfg.n_ttile * 64], F32)

        def rope_tab(sb, tt, nbc):
            """[128, nbc(bcast), 64] view of table column block tt."""
            ap = sb.ap()
            if nbc == 1:
                return ap[:, tt * 64:(tt + 1) * 64]
            return bass.AP(tensor=ap.tensor, offset=ap.offset + tt * 64,
                           ap=[list(ap.ap[0]), [0, nbc], [1, 64]])

        mask_sb = nc.alloc_sbuf_tensor("mask_sb", [128, NH * 128], BF16)
        ident = nc.alloc_sbuf_tensor("ident_sb", [128, 128], BF16)
        ones_sb = nc.alloc_sbuf_tensor("ones_sb", [128, 128], BF16)

        with tc.tile_pool(name="persist", bufs=1) as persist, \
             tc.tile_pool(name="weights", bufs=1) as wpool:
            # resident weights
            wq_sb = wpool.tile([128, cfg.n_dtile, QN], BF16, tag="wq")
            wkv_sb = wpool.tile([128, cfg.n_dtile, 2 * H], BF16, tag="wkv")

            # attention operand stores (full T)
            qT_sb = persist.tile([128, NH, T], BF16, tag="qT")
            kT_sb = persist.tile([128, T], BF16, tag="kT")
            vb_sb = persist.tile([128, cfg.n_ttile, H], BF16, tag="vb")

            # ---------------- projection + RoPE + transposes ----------------
            with tc.tile_pool(name="proj_sb", bufs=4) as psb, \
                 tc.tile_pool(name="rope_sb", bufs=3) as rsb, \
                 tc.tile_pool(name="stage_sb", bufs=3) as ssb, \
                 tc.tile_pool(name="proj_ps", bufs=2, space="PSUM") as pps, \
                 tc.tile_pool(name="tr_ps", bufs=2, space="PSUM") as tps:
                def load_x(tt, steps=None, eng=None):
                    x_sb = psb.tile([128, cfg.n_dtile, 128], BF16, tag="x")
                    base = max(cfg.n_dtile // 4, 1)
                    sched = list(steps or [])
                    q4 = 0
                    while q4 < cfg.n_dtile:
                        step = min(sched.pop(0) if sched else base,
                                   cfg.n_dtile - q4)
                        (eng or nc.sync).dma_start(
                            out=x_sb[:, q4:q4 + step, :],
                            in_=x_d.ap()[tt, :, q4 * 128:(q4 + step) * 128])
                        q4 += step
                    return x_sb

                # startup order: x0, then weights chunk-interleaved, then consts
                x_pre = load_x(0, steps=[2, 2, 4], eng=nc.scalar)
                wsched = [1, 1, 2] if cfg.n_dtile >= 8 else []
                kq = 0
                while kq < cfg.n_dtile:
                    step = min(wsched.pop(0) if wsched else 4,
                               cfg.n_dtile - kq)
                    nc.sync.dma_start(
                        out=wq_sb[:, kq:kq + step, :],
                        in_=wq_d.ap()[:, kq * QN:(kq + step) * QN])
                    nc.sync.dma_start(
                        out=wkv_sb[:, kq:kq + step, :],
                        in_=wkv_d.ap()[:, kq * 2 * H:(kq + step) * 2 * H])
                    kq += step
                nc.scalar.dma_start(out=cos_sb.ap(), in_=cos_d.ap())
                nc.scalar.dma_start(out=sin_sb.ap(), in_=sin_d.ap())
                nc.scalar.dma_start(out=mask_sb.ap(), in_=mask_d.ap())
                make_identity(nc, ident.ap())
                nc.vector.memset(ones_sb.ap(), 1.0)

                def proj_epilogue(tt, ps_q, ps_kv):
                    # --- RoPE on q (all NH heads at once via strided views) ---
                    q3 = ps_q.rearrange("p (g x) -> p g x", g=NH)
                    cosb = rope_tab(cos_sb, tt, NH)
                    sinb = rope_tab(sin_sb, tt, NH)
                    t1 = rsb.tile([128, NH, H], F32, tag="t1")
                    t2 = rsb.tile([128, NH, H], F32, tag="t2")
                    nc.vector.tensor_mul(t1[:, :, 0:64], q3[:, :, 0:64], cosb)
                    nc.vector.tensor_mul(t1[:, :, 64:128], q3[:, :, 64:128], cosb)
                    nc.vector.scalar_tensor_tensor(
                        out=t2[:, :, 0:64], in0=q3[:, :, 64:128], scalar=-1.0,
                        in1=sinb, op0=OP.mult, op1=OP.mult)
                    nc.vector.tensor_mul(t2[:, :, 64:128], q3[:, :, 0:64], sinb)
                    qrot = rsb.tile([128, QN], BF16, tag="qrot")
                    nc.vector.tensor_add(
                        qrot, t1.rearrange("p g x -> p (g x)"),
                        t2.rearrange("p g x -> p (g x)"))

                    # --- RoPE on k (fp32 result kept for the cache output) ---
                    cos1 = rope_tab(cos_sb, tt, 1)
                    sin1 = rope_tab(sin_sb, tt, 1)
                    t1k = rsb.tile([128, H], F32, tag="t1k")
                    t2k = rsb.tile([128, H], F32, tag="t2k")
                    nc.vector.tensor_mul(t1k[:, 0:64], ps_kv[:, 0:64], cos1)
                    nc.vector.tensor_mul(t1k[:, 64:128], ps_kv[:, 64:128], cos1)
                    nc.vector.scalar_tensor_tensor(
                        out=t2k[:, 0:64], in0=ps_kv[:, 64:128], scalar=-1.0,
                        in1=sin1, op0=OP.mult, op1=OP.mult)
                    nc.vector.tensor_mul(t2k[:, 64:128], ps_kv[:, 0:64], sin1)
                    kf = ssb.tile([128, H], F32, tag="kf")
                    nc.vector.tensor_add(kf, t1k, t2k)
                    nc.sync.dma_start(
                        out=k_out.ap()[tt * 128:(tt + 1) * 128, :], in_=kf)
                    kb = ssb.tile([128, H], BF16, tag="kb")
                    nc.vector.tensor_copy(kb, kf)

                    # --- v: copy out fp32 + keep bf16 for PV ---
                    vf = ssb.tile([128, H], F32, tag="vf")
                    nc.vector.tensor_copy(vf, ps_kv[:, H:2 * H])
                    nc.sync.dma_start(
                        out=v_out.ap()[tt * 128:(tt + 1) * 128, :], in_=vf)
                    nc.vector.tensor_copy(vb_sb[:, tt, :], ps_kv[:, H:2 * H])

                    # --- transposes into [h, t] attention layout ---
                    for g in range(NH):
                        ptr = tps.tile([128, 128], BF16, tag="ptr")
                        nc.tensor.transpose(
                            ptr, qrot[:, g * H:(g + 1) * H], ident.ap())
                        nc.vector.tensor_copy(
                            qT_sb[:, g, tt * 128:(tt + 1) * 128], ptr)
                    ptrk = tps.tile([128, 128], BF16, tag="ptr")
                    nc.tensor.transpose(ptrk, kb, ident.ap())
                    nc.vector.tensor_copy(kT_sb[:, tt * 128:(tt + 1) * 128], ptrk)

                def proj_mms(x_sb, ps_q, ps_kv, kk0, kk1):
                    for kk in range(kk0, kk1):
                        st = kk == 0
                        sp = kk == cfg.n_dtile - 1
                        nc.tensor.matmul(ps_q, x_sb[:, kk, :], wq_sb[:, kk, :],
                                         start=st, stop=sp)
                        nc.tensor.matmul(ps_kv, x_sb[:, kk, :], wkv_sb[:, kk, :],
                                         start=st, stop=sp)

                for tt in range(cfg.n_ttile):
                    x_sb = x_pre if tt == 0 else load_x(tt)
                    ps_q = pps.tile([128, QN], F32, tag="ps_q")
                    ps_kv = pps.tile([128, 2 * H], F32, tag="ps_kv")
                    proj_mms(x_sb, ps_q, ps_kv, 0, cfg.n_dtile)
                    proj_epilogue(tt, ps_q, ps_kv)

            # ---------------- attention + O projection, per sequence --------
            wo_sb = wpool.tile([128, NH, cfg.n_dtile, 128], BF16, tag="wo")
            nc.sync.dma_start(out=wo_sb, in_=wo_d.ap())

            scale = float(H) ** -0.5
            with tc.tile_pool(name="attn_sb", bufs=8) as asb, \
                 tc.tile_pool(name="outT_sb", bufs=2) as osb, \
                 tc.tile_pool(name="norm_sb", bufs=3) as nsb, \
                 tc.tile_pool(name="ostage_sb", bufs=6) as ostg, \
                 tc.tile_pool(name="sT_ps", bufs=3, space="PSUM") as sps, \
                 tc.tile_pool(name="outT_ps", bufs=2, space="PSUM") as ops, \
                 tc.tile_pool(name="z_ps", bufs=1, space="PSUM") as zps, \
                 tc.tile_pool(name="o_ps", bufs=2, space="PSUM") as oops:
                GW = NH * 128  # all heads side by side in the moving dim
                tq_per_chunk = CHUNK // 128

                pending = []  # (s, jc, dt0, dt1, outT) o-proj slices to weave

                def oproj_slice(s, jc, dt0, dt1, outT):
                    # O projection slice (wo stationary -> oT output [D, T];
                    # host transposes once at the end). Slices are woven
                    # between q-tiles as PE filler while ACT runs exp.
                    for dt in range(dt0, dt1):
                        ps_o = oops.tile([128, CHUNK], F32, tag="pso",
                                         name=f"pso_{s}_{jc}_{dt}")
                        for g in range(NH):
                            nc.tensor.matmul(
                                ps_o, wo_sb[:, g, dt, :],
                                outT[:, g, jc * CHUNK:(jc + 1) * CHUNK],
                                start=(g == 0), stop=(g == NH - 1))
                        o_sb = ostg.tile([128, CHUNK], BF16, tag="osb")
                        nc.vector.tensor_copy(o_sb, ps_o)
                        col0 = s * SQ + jc * CHUNK
                        nc.sync.dma_start(
                            out=o_out.ap()[dt * 128:(dt + 1) * 128,
                                           col0:col0 + CHUNK],
                            in_=o_sb)

                n_slice = max(cfg.n_dtile // 4, 1)  # dt per woven slice

                for s in range(cfg.n_seq):
                    outT = osb.tile([128, NH, SQ], BF16, tag="outT")
                    for tq in range(cfg.tt_per_seq):
                        if pending:
                            oproj_slice(*pending.pop(0))
                        q0 = s * SQ + tq * 128
                        # moving operand: 128 q-cols of every head (strided)
                        qmov = qT_sb[:, :, q0:q0 + 128]
                        ps_out = ops.tile([128, GW], F32, tag="po")
                        ps_z = zps.tile([128, GW], F32, tag="pz")
                        n_i = tq + 1
                        n_grp = (n_i + 3) // 4
                        grp = []   # pt tiles of the current group of <=4
                        gi = 0     # group index
                        for i in range(tq + 1):
                            st_g = s * cfg.tt_per_seq + i
                            ps_s = sps.tile([128, GW], F32, tag="ps")
                            nc.tensor.matmul(
                                ps_s.rearrange("p (g x) -> p g x", g=NH),
                                kT_sb[:, st_g * 128:(st_g + 1) * 128],
                                qmov, start=True, stop=True)
                            pt = asb.tile([128, GW], BF16, tag="pt")
                            nc.scalar.activation(pt, ps_s, AF.Exp, scale=scale)
                            if i == tq:  # diagonal: zero the causal upper part
                                ptm = asb.tile([128, GW], BF16, tag="pt")
                                nc.vector.tensor_mul(ptm, pt, mask_sb.ap())
                                pt = ptm
                            nc.tensor.matmul(
                                ps_out, vb_sb[:, st_g, :], pt,
                                start=(i == 0), stop=(i == tq))
                            # Z: pre-combine groups of 4 exp tiles on DVE,
                            # then one ones-matmul per group (saves 2/3 of
                            # the Z streaming on the PE)
                            grp.append(pt)
                            if len(grp) == 4 or i == tq:
                                while len(grp) > 1:
                                    nxt = []
                                    for a in range(0, len(grp) - 1, 2):
                                        zs = asb.tile([128, GW], BF16,
                                                      tag="zs",
                                                      name=f"zs_{s}_{tq}_{i}_{a}")
                                        nc.vector.tensor_add(
                                            zs, grp[a], grp[a + 1])
                                        nxt.append(zs)
                                    if len(grp) % 2:
                                        nxt.append(grp[-1])
                                    grp = nxt
                                nc.tensor.matmul(
                                    ps_z, ones_sb.ap(), grp[0],
                                    start=(gi == 0), stop=(gi == n_grp - 1))
                                grp = []
                                gi += 1
                        rz = nsb.tile([128, GW], F32, tag="rz")
                        nc.vector.reciprocal_approx_fast(out=rz, in_=ps_z)
                        nc.vector.tensor_mul(
                            outT[:, :, tq * 128:(tq + 1) * 128],
                            ps_out.rearrange("p (g x) -> p g x", g=NH),
                            rz.rearrange("p (g x) -> p g x", g=NH))
                        if (tq + 1) % tq_per_chunk == 0:
                            jc = tq // tq_per_chunk
                            for dt0 in range(0, cfg.n_dtile, n_slice):
                                pending.append(
                                    (s, jc, dt0, dt0 + n_slice, outT))
                for item in pending:  # drain the tail (last chunks)
                    oproj_slice(*item)

    nc.compile()
    return nc


# ---------------------------------------------------------------------------
# host side
# ---------------------------------------------------------------------------

def make_in_maps(cfg: Cfg, x, w_q, w_k, w_v, w_o, positions, n_cores=NCORES):
    """Per-core input maps. x:[T,D] f32, w_q:[D,N,H], w_k/w_v:[D,K,H],
    w_o:[N,H,D], positions:[T] int32."""
    bf16 = ml_dtypes.bfloat16
    D, T, NH, H = cfg.D, cfg.T, cfg.NH, cfg.H
    nt, nd, DC = cfg.n_ttile, cfg.n_dtile, cfg.DC
    # x swizzled to the exact SBUF layout: [tt, p(=d within tile), kk, t]
    xt = np.ascontiguousarray(
        x.reshape(nt, 128, nd, 128).transpose(0, 3, 2, 1)
         .reshape(nt, 128, nd * 128)).astype(bf16)
    half = H // 2
    inv_freq = (10000.0 ** (-np.arange(half, dtype=np.float64) * 2.0 / H))
    ang = positions.astype(np.float64)[:, None] * inv_freq[None, :]
    cosm = np.ascontiguousarray(
        np.cos(ang).astype(np.float32).reshape(nt, 128, 64)
        .transpose(1, 0, 2).reshape(128, nt * 64))
    sinm = np.ascontiguousarray(
        np.sin(ang).astype(np.float32).reshape(nt, 128, 64)
        .transpose(1, 0, 2).reshape(128, nt * 64))
    maskS = np.tile(
        np.where(np.arange(128)[:, None] <= np.arange(128)[None, :],
                 1.0, 0.0), (1, NH)).astype(bf16)

    def sw_w(w):  # [D, n] -> [p, kk, n] -> [128, kk*n]
        n = w.shape[1]
        return np.ascontiguousarray(
            w.reshape(nd, 128, n).transpose(1, 0, 2).reshape(128, nd * n))

    in_maps = []
    for c in range(n_cores):
        wq_c = sw_w(w_q[:, c * NH:(c + 1) * NH, :].reshape(D, NH * H)).astype(bf16)
        wkv_c = sw_w(np.concatenate(
            [w_k[:, c, :], w_v[:, c, :]], axis=1)).astype(bf16)
        wo_c = np.ascontiguousarray(
            w_o[c * NH:(c + 1) * NH].reshape(NH, H, nd, 128)
            .transpose(1, 0, 2, 3).reshape(128, NH * nd * 128)).astype(bf16)
        in_maps.append(dict(xt=xt, wq=wq_c, wkv=wkv_c, wo=wo_c,
                            cosm=cosm, sinm=sinm, maskS=maskS))
    return in_maps


_NC_CACHE = {}
_RUN_EXTRA = {}   # test harness can set e.g. {"trace": True}
_LAST_RES = {}    # test harness reads back the BassKernelResults


def kernel(x, k_cache, v_cache, w_q, w_k, w_v, w_o,
           block_tables, positions, slot_mapping, seq_lens):
    x = np.asarray(x); w_q = np.asarray(w_q); w_k = np.asarray(w_k)
    w_v = np.asarray(w_v); w_o = np.asarray(w_o)
    positions = np.asarray(positions); slot_mapping = np.asarray(slot_mapping)
    k_cache = np.asarray(k_cache); v_cache = np.asarray(v_cache)

    cfg = Cfg()
    T, D = x.shape
    K = w_k.shape[1]
    assert (T, D) == (cfg.T, cfg.D) and K == NCORES

    if "nc" not in _NC_CACHE:
        _NC_CACHE["nc"] = build_kernel(cfg)
    nc = _NC_CACHE["nc"]

    in_maps = make_in_maps(cfg, x, w_q, w_k, w_v, w_o, positions)
    res = bass_utils.run_bass_kernel_spmd(
        nc, in_maps, core_ids=list(range(NCORES)), **_RUN_EXTRA)
    _LAST_RES["res"] = res

    H = cfg.H
    ot = np.zeros((D, T), np.float32)
    k_full = np.empty((T, K, H), np.float32)
    v_full = np.empty((T, K, H), np.float32)
    for c in range(NCORES):
        r = res.results[c]
        ot += r["ot_part"].astype(np.float32)
        k_full[:, c, :] = r["k_out"]
        v_full[:, c, :] = r["v_out"]
    o = np.ascontiguousarray(ot.T)

    kc = k_cache.reshape(-1, K, H).copy()
    vc = v_cache.reshape(-1, K, H).copy()
    kc[slot_mapping] = k_full
    vc[slot_mapping] = v_full
    return (kc.reshape(k_cache.shape).astype(np.float32),
            vc.reshape(v_cache.shape).astype(np.float32),
            o)


# revision 44
# speedup vs baseline: 1.3641x; 1.0022x over previous
"""Paged-attention prefill kernel for Trainium2, tensor-parallel over heads on 8 cores.

Reference semantics (see problem): full prefill GQA attention with RoPE and a
paged KV cache. B=4 seqs x 1024 tokens, D=4096, 32 q heads / 8 kv heads, H=128.

Sharding: core c owns q heads 4c..4c+3 and kv head c. x is replicated (passed
pre-transposed as xT), w_q/w_o sharded on N, w_k/w_v on K. Each core computes
its 4 heads' attention and a partial O projection [T, D]; the host sums the 8
partials. k/v (post-RoPE) are returned per-core in token order; the host
scatters them into the paged caches via slot_mapping.

Device layout choices:
  - projections produce q/k/v in [t, h] layout (moving operand = weights,
    N=512/256), RoPE applied with free-dim strided views (no partition moves),
    then PE-transposes give qT/kT in [h, t] for attention.
  - scores are computed transposed: per 128-token q-tile, ONE matmul produces
    sT [s-tile, 4 heads x 128 q] with the 4 heads side by side in the moving
    dim (strided view of qT), so each LDWEIGHTS serves 4 heads' work. exp
    needs no max subtraction (scores ~ N(0,1), no overflow); causality: s-tiles
    above the diagonal are skipped, the diagonal tile is masked by a 0/1
    multiply on DVE after exp.
  - softmax denominator Z = ones[128,128]^T @ p accumulated in PSUM (M=128 so
    it pipelines like a normal matmul; rows are replicated so no partition
    broadcast is needed), then outT *= reciprocal_approx_fast(Z).
  - O projection keeps w_o stationary (reused across both 512-token moving
    chunks) and emits oT [D, T] partials in bf16; the host sums 8 partials in
    fp32 and transposes once.
  - all inputs are host-swizzled into the exact SBUF layouts so every DMA is
    partition-contiguous; leading x/weight DMAs are split fine-grained and
    spread across HWDGE queues so the first matmul starts at ~10us.
Compute dtype bf16 (fp32 PSUM accumulation).
"""
import numpy as np
import ml_dtypes

import concourse.bass as bass
import concourse.bacc as bacc
import concourse.mybir as mybir
import concourse.tile as tile
from concourse import bass_utils
from concourse.masks import make_identity

F32 = mybir.dt.float32
BF16 = mybir.dt.bfloat16
AF = mybir.ActivationFunctionType
OP = mybir.AluOpType

NCORES = 8


class Cfg:
    def __init__(self, D=4096, T=4096, SQ=1024, NH=4, CHUNK=512):
        self.D, self.T, self.SQ, self.NH, self.CHUNK = D, T, SQ, NH, CHUNK
        self.H = 128
        self.n_dtile = D // 128
        self.n_ttile = T // 128
        self.n_seq = T // SQ
        self.tt_per_seq = SQ // 128
        self.st_per_chunk = CHUNK // 128
        self.chunks_per_seq = SQ // CHUNK
        self.DC = D // 512  # d-chunks for O projection
        assert SQ % CHUNK == 0 and CHUNK % 128 == 0 and D % 512 == 0


def build_kernel(cfg: Cfg):
    D, T, SQ, NH, CHUNK = cfg.D, cfg.T, cfg.SQ, cfg.NH, cfg.CHUNK
    H = cfg.H
    QN = NH * H  # q projection width per core (4 heads -> 512)

    nc = bacc.Bacc("TRN2", target_bir_lowering=False, debug=False)

    # ---- DRAM I/O (per-core), all pre-swizzled on host for contiguous DMA ----
    x_d = nc.dram_tensor("xt", [cfg.n_ttile, 128, cfg.n_dtile * 128], BF16,
                         kind="ExternalInput")
    wq_d = nc.dram_tensor("wq", [128, cfg.n_dtile * QN], BF16,
                          kind="ExternalInput")
    wkv_d = nc.dram_tensor("wkv", [128, cfg.n_dtile * 2 * H], BF16,
                           kind="ExternalInput")
    wo_d = nc.dram_tensor("wo", [128, NH * cfg.DC * 512], BF16,
                          kind="ExternalInput")
    cos_d = nc.dram_tensor("cosm", [128, cfg.n_ttile * 64], F32,
                           kind="ExternalInput")
    sin_d = nc.dram_tensor("sinm", [128, cfg.n_ttile * 64], F32,
                           kind="ExternalInput")
    mask_d = nc.dram_tensor("maskS", [128, NH * 128], BF16,
                            kind="ExternalInput")

    k_out = nc.dram_tensor("k_out", [T, H], F32, kind="ExternalOutput")
    v_out = nc.dram_tensor("v_out", [T, H], F32, kind="ExternalOutput")
    o_out = nc.dram_tensor("ot_part", [D, T], BF16, kind="ExternalOutput")

    with tile.TileContext(nc) as tc:
        # persistent constants in raw SBUF (concrete addresses -> manual
        # broadcast APs are safe)
        cos_sb = nc.alloc_sbuf_tensor("cos_sb", [128, cfg.n_ttile * 64], F32)
        sin_sb = nc.alloc_sbuf_tensor("sin_sb", [128, cfg.n_ttile * 64], F32)

        def rope_tab(sb, tt, nbc):
            """[128, nbc(bcast), 64] view of table column block tt."""
            ap = sb.ap()
            if nbc == 1:
                return ap[:, tt * 64:(tt + 1) * 64]
            return bass.AP(tensor=ap.tensor, offset=ap.offset + tt * 64,
                           ap=[list(ap.ap[0]), [0, nbc], [1, 64]])

        mask_sb = nc.alloc_sbuf_tensor("mask_sb", [128, NH * 128], BF16)
        ident = nc.alloc_sbuf_tensor("ident_sb", [128, 128], BF16)
        ones_sb = nc.alloc_sbuf_tensor("ones_sb", [128, 128], BF16)

        with tc.tile_pool(name="persist", bufs=1) as persist, \
             tc.tile_pool(name="weights", bufs=1) as wpool:
            # resident weights
            wq_sb = wpool.tile([128, cfg.n_dtile, QN], BF16, tag="wq")
            wkv_sb = wpool.tile([128, cfg.n_dtile, 2 * H], BF16, tag="wkv")

            # attention operand stores (full T)
            qT_sb = persist.tile([128, NH, T], BF16, tag="qT")
            kT_sb = persist.tile([128, T], BF16, tag="kT")
            vb_sb = persist.tile([128, cfg.n_ttile, H], BF16, tag="vb")

            # ---------------- projection + RoPE + transposes ----------------
            with tc.tile_pool(name="proj_sb", bufs=4) as psb, \
                 tc.tile_pool(name="rope_sb", bufs=3) as rsb, \
                 tc.tile_pool(name="stage_sb", bufs=3) as ssb, \
                 tc.tile_pool(name="proj_ps", bufs=2, space="PSUM") as pps, \
                 tc.tile_pool(name="tr_ps", bufs=2, space="PSUM") as tps:
                def load_x(tt, steps=None, eng=None):
                    x_sb = psb.tile([128, cfg.n_dtile, 128], BF16, tag="x")
                    base = max(cfg.n_dtile // 4, 1)
                    sched = list(steps or [])
                    q4 = 0
                    while q4 < cfg.n_dtile:
                        step = min(sched.pop(0) if sched else base,
                                   cfg.n_dtile - q4)
                        (eng or nc.sync).dma_start(
                            out=x_sb[:, q4:q4 + step, :],
                            in_=x_d.ap()[tt, :, q4 * 128:(q4 + step) * 128])
                        q4 += step
                    return x_sb

                # startup order: x0, then weights chunk-interleaved, then consts
                x_pre = load_x(0, steps=[2, 2, 4], eng=nc.scalar)
                wsched = [1, 1, 2] if cfg.n_dtile >= 8 else []
                kq = 0
                while kq < cfg.n_dtile:
                    step = min(wsched.pop(0) if wsched else 4,
                               cfg.n_dtile - kq)
                    nc.sync.dma_start(
                        out=wq_sb[:, kq:kq + step, :],
                        in_=wq_d.ap()[:, kq * QN:(kq + step) * QN])
                    nc.sync.dma_start(
                        out=wkv_sb[:, kq:kq + step, :],
                        in_=wkv_d.ap()[:, kq * 2 * H:(kq + step) * 2 * H])
                    kq += step
                nc.scalar.dma_start(out=cos_sb.ap(), in_=cos_d.ap())
                nc.scalar.dma_start(out=sin_sb.ap(), in_=sin_d.ap())
                nc.scalar.dma_start(out=mask_sb.ap(), in_=mask_d.ap())
                make_identity(nc, ident.ap())
                nc.vector.memset(ones_sb.ap(), 1.0)

                def proj_epilogue(tt, ps_q, ps_kv):
                    # --- RoPE on q (all NH heads at once via strided views) ---
                    q3 = ps_q.rearrange("p (g x) -> p g x", g=NH)
                    cosb = rope_tab(cos_sb, tt, NH)
                    sinb = rope_tab(sin_sb, tt, NH)
                    t1 = rsb.tile([128, NH, H], F32, tag="t1")
                    t2 = rsb.tile([128, NH, H], F32, tag="t2")
                    nc.vector.tensor_mul(t1[:, :, 0:64], q3[:, :, 0:64], cosb)
                    nc.vector.tensor_mul(t1[:, :, 64:128], q3[:, :, 64:128], cosb)
                    nc.vector.scalar_tensor_tensor(
                        out=t2[:, :, 0:64], in0=q3[:, :, 64:128], scalar=-1.0,
                        in1=sinb, op0=OP.mult, op1=OP.mult)
                    nc.vector.tensor_mul(t2[:, :, 64:128], q3[:, :, 0:64], sinb)
                    qrot = rsb.tile([128, QN], BF16, tag="qrot")
                    nc.vector.tensor_add(
                        qrot, t1.rearrange("p g x -> p (g x)"),
                        t2.rearrange("p g x -> p (g x)"))

                    # --- RoPE on k (fp32 result kept for the cache output) ---
                    cos1 = rope_tab(cos_sb, tt, 1)
                    sin1 = rope_tab(sin_sb, tt, 1)
                    t1k = rsb.tile([128, H], F32, tag="t1k")
                    t2k = rsb.tile([128, H], F32, tag="t2k")
                    nc.vector.tensor_mul(t1k[:, 0:64], ps_kv[:, 0:64], cos1)
                    nc.vector.tensor_mul(t1k[:, 64:128], ps_kv[:, 64:128], cos1)
                    nc.vector.scalar_tensor_tensor(
                        out=t2k[:, 0:64], in0=ps_kv[:, 64:128], scalar=-1.0,
                        in1=sin1, op0=OP.mult, op1=OP.mult)
                    nc.vector.tensor_mul(t2k[:, 64:128], ps_kv[:, 0:64], sin1)
                    kf = ssb.tile([128, H], F32, tag="kf")
                    nc.vector.tensor_add(kf, t1k, t2k)
                    nc.sync.dma_start(
                        out=k_out.ap()[tt * 128:(tt + 1) * 128, :], in_=kf)
                    kb = ssb.tile([128, H], BF16, tag="kb")
                    nc.vector.tensor_copy(kb, kf)

                    # --- v: copy out fp32 + keep bf16 for PV ---
                    vf = ssb.tile([128, H], F32, tag="vf")
                    nc.vector.tensor_copy(vf, ps_kv[:, H:2 * H])
                    nc.sync.dma_start(
                        out=v_out.ap()[tt * 128:(tt + 1) * 128, :], in_=vf)
                    nc.vector.tensor_copy(vb_sb[:, tt, :], ps_kv[:, H:2 * H])

                    # --- transposes into [h, t] attention layout ---
                    for g in range(NH):
                        ptr = tps.tile([128, 128], BF16, tag="ptr")
                        nc.tensor.transpose(
                            ptr, qrot[:, g * H:(g + 1) * H], ident.ap())
                        nc.vector.tensor_copy(
                            qT_sb[:, g, tt * 128:(tt + 1) * 128], ptr)
                    ptrk = tps.tile([128, 128], BF16, tag="ptr")
                    nc.tensor.transpose(ptrk, kb, ident.ap())
                    nc.vector.tensor_copy(kT_sb[:, tt * 128:(tt + 1) * 128], ptrk)

                def proj_mms(x_sb, ps_q, ps_kv, kk0, kk1):
                    for kk in range(kk0, kk1):
                        st = kk == 0
                        sp = kk == cfg.n_dtile - 1
                        nc.tensor.matmul(ps_q, x_sb[:, kk, :], wq_sb[:, kk, :],
                                         start=st, stop=sp)
                        nc.tensor.matmul(ps_kv, x_sb[:, kk, :], wkv_sb[:, kk, :],
                                         start=st, stop=sp)

                for tt in range(cfg.n_ttile):
                    x_sb = x_pre if tt == 0 else load_x(tt)
                    ps_q = pps.tile([128, QN], F32, tag="ps_q")
                    ps_kv = pps.tile([128, 2 * H], F32, tag="ps_kv")
                    proj_mms(x_sb, ps_q, ps_kv, 0, cfg.n_dtile)
                    proj_epilogue(tt, ps_q, ps_kv)

            # ---------------- attention + O projection, per sequence --------
            wo_sb = wpool.tile([128, NH, cfg.n_dtile, 128], BF16, tag="wo")
            nc.sync.dma_start(out=wo_sb, in_=wo_d.ap())

            scale = float(H) ** -0.5
            with tc.tile_pool(name="attn_sb", bufs=10) as asb, \
                 tc.tile_pool(name="outT_sb", bufs=2) as osb, \
                 tc.tile_pool(name="norm_sb", bufs=3) as nsb, \
                 tc.tile_pool(name="ostage_sb", bufs=6) as ostg, \
                 tc.tile_pool(name="sT_ps", bufs=3, space="PSUM") as sps, \
                 tc.tile_pool(name="outT_ps", bufs=2, space="PSUM") as ops, \
                 tc.tile_pool(name="z_ps", bufs=1, space="PSUM") as zps, \
                 tc.tile_pool(name="o_ps", bufs=2, space="PSUM") as oops:
                GW = NH * 128  # all heads side by side in the moving dim
                tq_per_chunk = CHUNK // 128

                pending = []  # (s, jc, dt0, dt1, outT) o-proj slices to weave

                def oproj_slice(s, jc, dt0, dt1, outT):
                    # O projection slice (wo stationary -> oT output [D, T];
                    # host transposes once at the end). Slices are woven
                    # between q-tiles as PE filler while ACT runs exp.
                    for dt in range(dt0, dt1):
                        ps_o = oops.tile([128, CHUNK], F32, tag="pso",
                                         name=f"pso_{s}_{jc}_{dt}")
                        for g in range(NH):
                            nc.tensor.matmul(
                                ps_o, wo_sb[:, g, dt, :],
                                outT[:, g, jc * CHUNK:(jc + 1) * CHUNK],
                                start=(g == 0), stop=(g == NH - 1))
                        o_sb = ostg.tile([128, CHUNK], BF16, tag="osb")
                        nc.vector.tensor_copy(o_sb, ps_o)
                        col0 = s * SQ + jc * CHUNK
                        nc.sync.dma_start(
                            out=o_out.ap()[dt * 128:(dt + 1) * 128,
                                           col0:col0 + CHUNK],
                            in_=o_sb)

                n_slice = max(cfg.n_dtile // 4, 1)  # dt per woven slice

                for s in range(cfg.n_seq):
                    outT = osb.tile([128, NH, SQ], BF16, tag="outT")
                    for tq in range(cfg.tt_per_seq):
                        if pending:
                            oproj_slice(*pending.pop(0))
                        q0 = s * SQ + tq * 128
                        # moving operand: 128 q-cols of every head (strided)
                        qmov = qT_sb[:, :, q0:q0 + 128]
                        ps_out = ops.tile([128, GW], F32, tag="po")
                        ps_z = zps.tile([128, GW], F32, tag="pz")
                        n_i = tq + 1
                        n_grp = (n_i + 7) // 8
                        grp = []   # pt tiles of the current group of <=4
                        gi = 0     # group index
                        for i in range(tq + 1):
                            st_g = s * cfg.tt_per_seq + i
                            ps_s = sps.tile([128, GW], F32, tag="ps")
                            nc.tensor.matmul(
                                ps_s.rearrange("p (g x) -> p g x", g=NH),
                                kT_sb[:, st_g * 128:(st_g + 1) * 128],
                                qmov, start=True, stop=True)
                            pt = asb.tile([128, GW], BF16, tag="pt")
                            nc.scalar.activation(pt, ps_s, AF.Exp, scale=scale)
                            if i == tq:  # diagonal: zero the causal upper part
                                ptm = asb.tile([128, GW], BF16, tag="pt")
                                nc.vector.tensor_mul(ptm, pt, mask_sb.ap())
                                pt = ptm
                            nc.tensor.matmul(
                                ps_out, vb_sb[:, st_g, :], pt,
                                start=(i == 0), stop=(i == tq))
                            # Z: pre-combine groups of 4 exp tiles on DVE,
                            # then one ones-matmul per group (saves 2/3 of
                            # the Z streaming on the PE)
                            grp.append(pt)
                            if len(grp) == 8 or i == tq:
                                while len(grp) > 1:
                                    nxt = []
                                    for a in range(0, len(grp) - 1, 2):
                                        zs = asb.tile([128, GW], BF16,
                                                      tag="zs",
                                                      name=f"zs_{s}_{tq}_{i}_{a}")
                                        nc.vector.tensor_add(
                                            zs, grp[a], grp[a + 1])
                                        nxt.append(zs)
                                    if len(grp) % 2:
                                        nxt.append(grp[-1])
                                    grp = nxt
                                nc.tensor.matmul(
                                    ps_z, ones_sb.ap(), grp[0],
                                    start=(gi == 0), stop=(gi == n_grp - 1))
                                grp = []
                                gi += 1
                        rz = nsb.tile([128, GW], F32, tag="rz")
                        nc.vector.reciprocal_approx_fast(out=rz, in_=ps_z)
                        nc.vector.tensor_mul(
                            outT[:, :, tq * 128:(tq + 1) * 128],
                            ps_out.rearrange("p (g x) -> p g x", g=NH),
                            rz.rearrange("p (g x) -> p g x", g=NH))
                        if (tq + 1) % tq_per_chunk == 0:
                            jc = tq // tq_per_chunk
                            for dt0 in range(0, cfg.n_dtile, n_slice):
                                pending.append(
                                    (s, jc, dt0, dt0 + n_slice, outT))
                for item in pending:  # drain the tail (last chunks)
                    oproj_slice(*item)

    nc.compile()
    return nc


# ---------------------------------------------------------------------------
# host side
# ---------------------------------------------------------------------------

def make_in_maps(cfg: Cfg, x, w_q, w_k, w_v, w_o, positions, n_cores=NCORES):
    """Per-core input maps. x:[T,D] f32, w_q:[D,N,H], w_k/w_v:[D,K,H],
    w_o:[N,H,D], positions:[T] int32."""
    bf16 = ml_dtypes.bfloat16
    D, T, NH, H = cfg.D, cfg.T, cfg.NH, cfg.H
    nt, nd, DC = cfg.n_ttile, cfg.n_dtile, cfg.DC
    # x swizzled to the exact SBUF layout: [tt, p(=d within tile), kk, t]
    xt = np.ascontiguousarray(
        x.reshape(nt, 128, nd, 128).transpose(0, 3, 2, 1)
         .reshape(nt, 128, nd * 128)).astype(bf16)
    half = H // 2
    inv_freq = (10000.0 ** (-np.arange(half, dtype=np.float64) * 2.0 / H))
    ang = positions.astype(np.float64)[:, None] * inv_freq[None, :]
    cosm = np.ascontiguousarray(
        np.cos(ang).astype(np.float32).reshape(nt, 128, 64)
        .transpose(1, 0, 2).reshape(128, nt * 64))
    sinm = np.ascontiguousarray(
        np.sin(ang).astype(np.float32).reshape(nt, 128, 64)
        .transpose(1, 0, 2).reshape(128, nt * 64))
    maskS = np.tile(
        np.where(np.arange(128)[:, None] <= np.arange(128)[None, :],
                 1.0, 0.0), (1, NH)).astype(bf16)

    def sw_w(w):  # [D, n] -> [p, kk, n] -> [128, kk*n]
        n = w.shape[1]
        return np.ascontiguousarray(
            w.reshape(nd, 128, n).transpose(1, 0, 2).reshape(128, nd * n))

    in_maps = []
    for c in range(n_cores):
        wq_c = sw_w(w_q[:, c * NH:(c + 1) * NH, :].reshape(D, NH * H)).astype(bf16)
        wkv_c = sw_w(np.concatenate(
            [w_k[:, c, :], w_v[:, c, :]], axis=1)).astype(bf16)
        wo_c = np.ascontiguousarray(
            w_o[c * NH:(c + 1) * NH].reshape(NH, H, nd, 128)
            .transpose(1, 0, 2, 3).reshape(128, NH * nd * 128)).astype(bf16)
        in_maps.append(dict(xt=xt, wq=wq_c, wkv=wkv_c, wo=wo_c,
                            cosm=cosm, sinm=sinm, maskS=maskS))
    return in_maps


_NC_CACHE = {}
_RUN_EXTRA = {}   # test harness can set e.g. {"trace": True}
_LAST_RES = {}    # test harness reads back the BassKernelResults


def kernel(x, k_cache, v_cache, w_q, w_k, w_v, w_o,
           block_tables, positions, slot_mapping, seq_lens):
    x = np.asarray(x); w_q = np.asarray(w_q); w_k = np.asarray(w_k)
    w_v = np.asarray(w_v); w_o = np.asarray(w_o)
    positions = np.asarray(positions); slot_mapping = np.asarray(slot_mapping)
    k_cache = np.asarray(k_cache); v_cache = np.asarray(v_cache)

    cfg = Cfg()
    T, D = x.shape
    K = w_k.shape[1]
    assert (T, D) == (cfg.T, cfg.D) and K == NCORES

    if "nc" not in _NC_CACHE:
        _NC_CACHE["nc"] = build_kernel(cfg)
    nc = _NC_CACHE["nc"]

    in_maps = make_in_maps(cfg, x, w_q, w_k, w_v, w_o, positions)
    res = bass_utils.run_bass_kernel_spmd(
        nc, in_maps, core_ids=list(range(NCORES)), **_RUN_EXTRA)
    _LAST_RES["res"] = res

    H = cfg.H
    ot = np.zeros((D, T), np.float32)
    k_full = np.empty((T, K, H), np.float32)
    v_full = np.empty((T, K, H), np.float32)
    for c in range(NCORES):
        r = res.results[c]
        ot += r["ot_part"].astype(np.float32)
        k_full[:, c, :] = r["k_out"]
        v_full[:, c, :] = r["v_out"]
    o = np.ascontiguousarray(ot.T)

    kc = k_cache.reshape(-1, K, H).copy()
    vc = v_cache.reshape(-1, K, H).copy()
    kc[slot_mapping] = k_full
    vc[slot_mapping] = v_full
    return (kc.reshape(k_cache.shape).astype(np.float32),
            vc.reshape(v_cache.shape).astype(np.float32),
            o)
